# revision 3
# baseline (speedup 1.0000x reference)
"""DepthConditionedGWM kernel — self-contained, full-input contract.

Strategy: depth-dependent small parameters (wave coefficients, depth
embedding, fused gate biases, parent queries) are computed on host in
numpy; the heavy per-sample work (wave rotation, gating, small-K
attention, layernorm, gated skip) runs data-parallel on the 8
NeuronCores via a jitted per-shard function (pure data parallel over B,
params replicated). B is processed in pipelined chunks so host->device
transfer, device compute, and device->host readback overlap.

Falls back to a pure-numpy implementation on any device-path failure.
"""
import math
import numpy as np

B = 2048; H = 16; KL = 16; KR = 16; D = 64
K_MAX = 32; MAX_DEPTH = 12; DE = 32
NCORES = 8
NCHUNK = 4          # B is split into NCHUNK pipelined chunks
EPS = 1e-5


# ---------------------------------------------------------------- host math
def _sigmoid(x):
    out = np.empty_like(x)
    pos = x >= 0
    out[pos] = 1.0 / (1.0 + np.exp(-x[pos]))
    ex = np.exp(x[~pos])
    out[~pos] = ex / (1.0 + ex)
    return out


def _softplus(x):
    return np.log1p(np.exp(-np.abs(x))) + np.maximum(x, 0.0)


def _depth_vec(depth):
    inv = np.exp(np.arange(0, DE, 2, dtype=np.float32) * (-math.log(10000.0) / DE))
    return np.stack([np.sin(depth * inv), np.cos(depth * inv)], axis=1).reshape(-1).astype(np.float32)


def _host_params(glw, glb, grw, grb, pqb, qow, qob, sa, wf, wd, wp, depth):
    """All tiny depth-dependent parameters, computed exactly as the module does."""
    scale = np.float32(max(2 ** (MAX_DEPTH - depth), 1))
    alpha_d = _softplus(wd) / scale
    omega_d = wf / scale
    phi_d = wp + np.float32(depth * (math.pi / 4.0))
    decay = np.exp(-alpha_d * scale)
    pr = (decay * np.cos(omega_d * scale + phi_d)).astype(np.float32)   # (H,)
    pi_ = (decay * np.sin(omega_d * scale + phi_d)).astype(np.float32)  # (H,)
    dv = _depth_vec(depth)                                              # (DE,)
    cl = (dv @ glw[2 * D:] + glb).astype(np.float32)                    # (D,)
    cr = (dv @ grw[2 * D:] + grb).astype(np.float32)
    q_off = (dv @ qow + qob).reshape(K_MAX, D)
    k_parent = min(KL + KR, K_MAX)
    pq = (pqb[:k_parent] + q_off[:k_parent]).astype(np.float32)         # (32,D)
    sig_sa = np.float32(1.0 / (1.0 + math.exp(-float(sa))))
    return pr, pi_, cl, cr, pq, sig_sa


# ---------------------------------------------------------------- device path
_DEV_STATE = None


def _get_device_fn():
    """Build (once) the jitted per-shard function + device list."""
    global _DEV_STATE
    if _DEV_STATE is not None:
        return _DEV_STATE
    import jax
    import jax.numpy as jnp

    devs = [d for d in jax.devices() if d.platform != "cpu"]
    if len(devs) < NCORES:
        raise RuntimeError(f"need {NCORES} accelerator cores, got {len(devs)}")
    devs = devs[:NCORES]

    def shard_fn(fl, fr, pr, pi_, Wl, cl, Wr, cr, pq, gam, bet, spw, sig_sa):
        # wave rotation of right child (complex multiply on split halves)
        d2 = D // 2
        frl = fr[..., :d2]
        fri = fr[..., d2:]
        prb = pr[None, :, None, None]
        pib = pi_[None, :, None, None]
        rot = jnp.concatenate([prb * frl - pib * fri, pib * frl + prb * fri], axis=-1)
        # depth-conditioned gating (depth terms folded into cl/cr on host)
        lm = fl.mean(axis=2)
        rm = rot.mean(axis=2)
        gin = jnp.concatenate([lm, rm], axis=-1)
        zl = gin @ Wl + cl
        zr = gin @ Wr + cr
        g_l = 1.0 / (1.0 + jnp.exp(-zl))
        g_r = 1.0 / (1.0 + jnp.exp(-zr))
        bank = jnp.concatenate([fl * g_l[:, :, None, :], rot * g_r[:, :, None, :]], axis=2)
        # small-K attention with depth-offset parent queries
        logits = jnp.einsum('qd,bhkd->bhqk', pq, bank) / math.sqrt(D)
        logits = logits - logits.max(axis=-1, keepdims=True)
        ex = jnp.exp(logits)
        attn = ex / ex.sum(axis=-1, keepdims=True)
        parent = jnp.einsum('bhqk,bhkd->bhqd', attn, bank)
        # layernorm over D (biased variance, eps=1e-5)
        mu = parent.mean(axis=-1, keepdims=True)
        xc = parent - mu
        var = (xc * xc).mean(axis=-1, keepdims=True)
        pn = xc / jnp.sqrt(var + EPS) * gam + bet
        # gated skip from left mean
        skip = (lm @ spw)[:, :, None, :]
        return pn + sig_sa * skip

    jit_fn = jax.jit(shard_fn)
    _DEV_STATE = (jax, jit_fn, devs)
    return _DEV_STATE


def _device_kernel(fl, fr, pr, pi_, Wl, cl, Wr, cr, pq, gam, bet, spw, sig_sa):
    from concurrent.futures import ThreadPoolExecutor
    jax, jit_fn, devs = _get_device_fn()

    # replicate the tiny params on every core once
    small = [pr, pi_, Wl, cl, Wr, cr, pq, gam, bet, spw,
             np.asarray(sig_sa, dtype=np.float32)]
    small_d = [[jax.device_put(s, d) for s in small] for d in devs]

    bsz = B // NCORES            # per-core batch
    csz = bsz // NCHUNK          # per-core, per-chunk batch
    fl_v = fl.reshape(NCORES, bsz, H, KL, D)
    fr_v = fr.reshape(NCORES, bsz, H, KR, D)

    # Concurrent transfers multiplex the tunnel (I/O releases the GIL);
    # compute for each (chunk, core) is dispatched as soon as its inputs
    # are enqueued, and readback runs on the same pool, overlapping the
    # remaining uploads/compute.
    pool = ThreadPoolExecutor(max_workers=2 * NCORES)

    def _upload_and_run(ci):
        c, i = ci
        sl = slice(c * csz, (c + 1) * csz)
        fl_d = jax.device_put(np.ascontiguousarray(fl_v[i, sl]), devs[i])
        fr_d = jax.device_put(np.ascontiguousarray(fr_v[i, sl]), devs[i])
        return jax.device_get(jit_fn(fl_d, fr_d, *small_d[i]))

    jobs = [(c, i) for c in range(NCHUNK) for i in range(NCORES)]
    futs = [pool.submit(_upload_and_run, ci) for ci in jobs]

    out = np.empty((NCORES, bsz, H, K_MAX, D), dtype=np.float32)
    for (c, i), fut in zip(jobs, futs):
        sl = slice(c * csz, (c + 1) * csz)
        out[i, sl] = fut.result()
    pool.shutdown(wait=False)
    return out.reshape(B, H, K_MAX, D)


# ---------------------------------------------------------------- numpy fallback
def _numpy_kernel(fl, fr, pr, pi_, Wl, cl, Wr, cr, pq, gam, bet, spw, sig_sa):
    d2 = D // 2
    frl = fr[..., :d2]
    fri = fr[..., d2:]
    prb = pr[None, :, None, None]
    pib = pi_[None, :, None, None]
    rot = np.concatenate([prb * frl - pib * fri, pib * frl + prb * fri], axis=-1)

    lm = fl.mean(axis=2, dtype=np.float32)
    rm = rot.mean(axis=2, dtype=np.float32)
    BH = B * H
    lm2 = lm.reshape(BH, D)
    rm2 = rm.reshape(BH, D)
    zl = lm2 @ Wl[:D] + rm2 @ Wl[D:] + cl[None, :]
    zr = lm2 @ Wr[:D] + rm2 @ Wr[D:] + cr[None, :]
    g_l = _sigmoid(zl).reshape(B, H, 1, D)
    g_r = _sigmoid(zr).reshape(B, H, 1, D)

    bank = np.concatenate([fl * g_l, rot * g_r], axis=2)
    bank2 = bank.reshape(BH, KL + KR, D)
    logits = np.matmul(bank2, pq.T) * np.float32(1.0 / math.sqrt(D))
    logits = logits.transpose(0, 2, 1)
    logits -= logits.max(axis=-1, keepdims=True)
    ex = np.exp(logits)
    attn = ex / ex.sum(axis=-1, keepdims=True)
    parent = np.matmul(attn.astype(np.float32), bank2)

    mu = parent.mean(axis=-1, keepdims=True, dtype=np.float32)
    xc = parent - mu
    var = np.mean(xc * xc, axis=-1, keepdims=True, dtype=np.float32)
    pn = xc / np.sqrt(var + np.float32(EPS)) * gam + bet

    skip = (lm2 @ spw).reshape(B, H, 1, D)
    out = pn.reshape(B, H, K_MAX, D) + sig_sa * skip
    return out.astype(np.float32)


# ---------------------------------------------------------------- entry point
def kernel(f_left, f_right, gate_left_w, gate_left_b, gate_right_w, gate_right_b,
           parent_query_base, query_offset_w, query_offset_b, ln_gamma, ln_beta,
           skip_proj_w, skip_alpha, wave_freq, wave_damp, wave_phase, depth):
    fl = np.asarray(f_left, dtype=np.float32)
    fr = np.asarray(f_right, dtype=np.float32)
    glw = np.asarray(gate_left_w, dtype=np.float32)
    glb = np.asarray(gate_left_b, dtype=np.float32)
    grw = np.asarray(gate_right_w, dtype=np.float32)
    grb = np.asarray(gate_right_b, dtype=np.float32)
    pqb = np.asarray(parent_query_base, dtype=np.float32)
    qow = np.asarray(query_offset_w, dtype=np.float32)
    qob = np.asarray(query_offset_b, dtype=np.float32)
    gam = np.asarray(ln_gamma, dtype=np.float32)
    bet = np.asarray(ln_beta, dtype=np.float32)
    spw = np.asarray(skip_proj_w, dtype=np.float32)
    sa = np.float32(np.asarray(skip_alpha))
    wf = np.asarray(wave_freq, dtype=np.float32)
    wd = np.asarray(wave_damp, dtype=np.float32)
    wp = np.asarray(wave_phase, dtype=np.float32)
    depth = int(np.asarray(depth))

    pr, pi_, cl, cr, pq, sig_sa = _host_params(
        glw, glb, grw, grb, pqb, qow, qob, sa, wf, wd, wp, depth)
    Wl = np.ascontiguousarray(glw[:2 * D])   # (128, 64)
    Wr = np.ascontiguousarray(grw[:2 * D])

    args = (fl, fr, pr, pi_, Wl, cl, Wr, cr, pq, gam, bet, spw, sig_sa)
    if fl.shape == (B, H, KL, D) and fr.shape == (B, H, KR, D):
        try:
            return _device_kernel(*args)
        except Exception:
            pass
    return _numpy_kernel(*args)
